# revision 23
# baseline (speedup 1.0000x reference)
"""Trainium2 Bass kernel for nn_ConsistencyMaskFromBoxes.

Computes: loss = WEIGHT * mean(BCEWithLogits(seg_pred * eff, boxes_mask * eff))

Algorithm
---------
For effective images (not is_seg, has boxes), per-pixel BCE with a {0,1}
target t factorizes:
    bce = max(l,0) - l*t + log1p(exp(-|l|)) = softplus(l) - l*t
so  sum(bce) = sum(softplus(l)) - sum_{mask} l.

Host decomposes each image's box-union into DISJOINT rects (sweep line), so
mask(y,x) = sum_r rowhit[r,y]*colhit[r,x] exactly (no clamping needed), and
    sum_{mask} l = sum_r sum_x colhit[r,x] * (sum_y rowhit[r,y] * l[y,x])
The inner contraction over y is a matmul on the PE (rowhit as stationary),
accumulated over 5 row-tiles of 128 into PSUM [R, 640]; the outer dot with
colhit is one fused multiply+reduce on the vector engine. sum(softplus(l))
is one scalar-engine activation per image with accum output.

Device does the heavy [B,1,640,640] traffic; host does the tiny per-box
preprocessing and the final (few hundred floats) reduction in float64.

Sharding: data-parallel over batch, 2 images per core on 8 cores (SPMD,
per-core differences carried entirely by inputs).
"""

import math
import numpy as np
import ml_dtypes

import concourse.bass as bass
import concourse.bacc as bacc
import concourse.mybir as mybir
import concourse.tile as tile
from concourse.bass_utils import run_bass_kernel_spmd

WEIGHT = 0.1
B, M, H, W = 16, 256, 640, 640

# Both Exp and Ln live in the 'natural_log_exp_and_others' table set, but the
# table-load pass picks the first set containing each function, alternating
# sets and inserting a ~1.3us ACT_TABLE_LOAD before every activation. Blank
# all other sets (indices must be preserved — act_func_set_id is the index
# into act_info.json) so one load covers the whole kernel.
_ACT_TABLE_KEEP = "natural_log_exp_and_others"
_orig_get_activation_tables = None


def _patch_act_tables():
    global _orig_get_activation_tables
    if _orig_get_activation_tables is not None:
        return
    import concourse.hw_specs as hw_specs
    _orig_get_activation_tables = hw_specs.get_activation_tables

    def patched(arch):
        tabs = _orig_get_activation_tables(arch)
        if _ACT_TABLE_KEEP in tabs:
            tabs = {name: (fns if name == _ACT_TABLE_KEEP else set())
                    for name, fns in tabs.items()}
        return tabs

    hw_specs.get_activation_tables = patched
    bacc.get_activation_tables = patched
N_CORES = 8
IPC = B // N_CORES          # images per core
PT = 128                    # SBUF partitions
NT = H // PT                # row tiles per image (5)
SEG_DT = mybir.dt.bfloat16  # dtype for seg_pred on device
SEG_NP = ml_dtypes.bfloat16

_PROG_CACHE: dict[tuple, object] = {}

# test-harness hooks (ignored in normal use): set TRACE=True to profile the
# SPMD launch; the BassKernelResults lands in LAST_RESULT.
TRACE = False
LAST_RESULT = None


# ----------------------------------------------------------------- host prep

def _box_coords(bboxes: np.ndarray, h: int, w: int):
    """Integer box corners, bit-exact float32 math as the reference."""
    bb = bboxes.astype(np.float32)
    cx = bb[:, 0] * np.float32(w)
    cy = bb[:, 1] * np.float32(h)
    bw = bb[:, 2] * np.float32(w)
    bh = bb[:, 3] * np.float32(h)
    two = np.float32(2.0)
    x1 = np.clip(cx - bw / two, 0.0, w - 1).astype(np.int32)
    y1 = np.clip(cy - bh / two, 0.0, h - 1).astype(np.int32)
    x2 = np.clip(cx + bw / two, 0.0, w - 1).astype(np.int32)
    y2 = np.clip(cy + bh / two, 0.0, h - 1).astype(np.int32)
    return x1, y1, x2, y2


def _disjoint_rects(boxes):
    """boxes: list of (x1,y1,x2,y2) inclusive ints. Returns disjoint rects
    (x1,x2,y1,y2) inclusive whose union equals the union of the boxes."""
    if not boxes:
        return []
    edges = sorted(set([b[0] for b in boxes] + [b[2] + 1 for b in boxes]))
    slabs = []
    for i in range(len(edges) - 1):
        xs, xe = edges[i], edges[i + 1]
        active = sorted((b[1], b[3]) for b in boxes if b[0] <= xs and b[2] + 1 >= xe)
        ints = []
        for a, bb in active:
            if ints and a <= ints[-1][1] + 1:
                ints[-1][1] = max(ints[-1][1], bb)
            else:
                ints.append([a, bb])
        if ints:
            slabs.append((xs, xe, tuple(tuple(t) for t in ints)))
    merged = []
    for xs, xe, ints in slabs:
        if merged and merged[-1][1] == xs and merged[-1][2] == ints:
            merged[-1][1] = xe
        else:
            merged.append([xs, xe, ints])
    out = []
    for xs, xe, ints in merged:
        for a, bb in ints:
            out.append((xs, xe - 1, a, bb))
    return out


# ------------------------------------------------------------- device program

def _build_program(k_pad: int, n_chunks: int):
    """SPMD program for one core: IPC images, each with n_chunks groups of
    up to k_pad disjoint rects. Returns compiled Bacc."""
    V = IPC * n_chunks  # virtual (image, chunk) pairs
    _patch_act_tables()
    nc = bacc.Bacc("TRN2", target_bir_lowering=False, debug=False)

    # seg is host-transposed to [image, sbuf_partition, row_tile * col] so
    # each partition's DMA payload is one contiguous 6.4KB chunk
    seg = nc.dram_tensor("seg", [IPC, PT, NT * W], SEG_DT, kind="ExternalInput")
    rowhit = nc.dram_tensor("rowhit", [PT, V * NT * k_pad], SEG_DT,
                            kind="ExternalInput")
    colhit = nc.dram_tensor("colhit", [k_pad, V * W], SEG_DT,
                            kind="ExternalInput")
    out_d = nc.dram_tensor("out", [PT, IPC + V], mybir.dt.float32,
                           kind="ExternalOutput")

    AF = mybir.ActivationFunctionType
    OP = mybir.AluOpType

    with tile.TileContext(nc) as tc:
        with (
            tc.tile_pool(name="seg", bufs=2) as seg_pool,
            tc.tile_pool(name="small", bufs=1) as small_pool,
            tc.tile_pool(name="ch", bufs=2) as ch_pool,
            tc.tile_pool(name="scratch", bufs=1) as scratch_pool,
            tc.tile_pool(name="ps", bufs=2, space="PSUM") as psum_pool,
        ):
            # seg DMAs first: they feed the scalar-engine critical path.
            # Image 0 arrives in 3 chunks sized so each EXP chunk's data
            # lands just before the scalar engine gets to it — the first
            # EXP starts ~2.5us earlier and never stalls.
            I0_EDGES = [0, 448, 1664, NT * W]
            seg_ts = []
            rh = small_pool.tile([PT, V * NT * k_pad], SEG_DT, tag="rh")
            ch_t = ch_pool.tile([k_pad, V * W], SEG_DT, tag="ch")
            for i in range(IPC):
                seg_t = seg_pool.tile([PT, NT * W], SEG_DT, tag=f"seg{i}")
                if i == 0:
                    for lo, hi in zip(I0_EDGES[:-1], I0_EDGES[1:]):
                        nc.sync.dma_start(seg_t[:, lo:hi], seg[i][:, lo:hi])
                    nc.sync.dma_start(rh[:], rowhit[:])
                    nc.sync.dma_start(ch_t[:], colhit[:])
                else:
                    nc.sync.dma_start(seg_t[:], seg[i])
                seg_ts.append(seg_t)

            spacc = small_pool.tile([PT, IPC + V], mybir.dt.float32,
                                    tag="spacc")
            one_t = small_pool.tile([PT, 1], mybir.dt.float32, tag="one")
            nc.vector.memset(one_t[:], 1.0)
            nc.vector.memset(spacc[:], 0.0)

            for i in range(IPC):
                seg_t = seg_ts[i]
                # sum of softplus over the whole image, per partition:
                # softplus(l) = Ln(1 + Exp(l)); Exp and Ln share one act table.
                # Image 0's Exp is split so it can start on the first half.
                e_scr = scratch_pool.tile([PT, NT * W], mybir.dt.float32,
                                          tag="e_scr")
                sp_scr = scratch_pool.tile([PT, NT * W], SEG_DT, tag="sp_scr")
                chunks = (list(zip(I0_EDGES[:-1], I0_EDGES[1:])) if i == 0
                          else [(0, NT * W)])
                for (lo, hi) in chunks:
                    nc.scalar.activation(e_scr[:, lo:hi], seg_t[:, lo:hi],
                                         AF.Exp)
                nc.scalar.activation(sp_scr[:], e_scr[:], AF.Ln,
                                     bias=one_t[:, 0:1],
                                     accum_out=spacc[:, i:i + 1])

                for c in range(n_chunks):
                    v = i * n_chunks + c
                    ps = psum_pool.tile([k_pad, W], mybir.dt.float32, tag="ps")
                    for t in range(NT):
                        lhsT = rh[:, (v * NT + t) * k_pad:(v * NT + t + 1) * k_pad]
                        rhs = seg_t[:, t * W:(t + 1) * W]
                        nc.tensor.matmul(ps[:, 0:512], lhsT, rhs[:, 0:512],
                                         start=(t == 0), stop=(t == NT - 1))
                        nc.tensor.matmul(ps[:, 512:W], lhsT, rhs[:, 512:W],
                                         start=(t == 0), stop=(t == NT - 1))
                    mm_scr = scratch_pool.tile([k_pad, W], mybir.dt.float32,
                                               tag="mm_scr")
                    nc.vector.tensor_mul(mm_scr[:], ps[:],
                                         ch_t[:, v * W:(v + 1) * W])
                    nc.vector.tensor_reduce(
                        spacc[:k_pad, IPC + v:IPC + v + 1], mm_scr[:],
                        axis=mybir.AxisListType.X, op=OP.add)

            nc.scalar.dma_start(out_d[:], spacc[:])

    nc.compile()
    return nc


def _get_program(k_pad: int, n_chunks: int):
    key = (k_pad, n_chunks)
    if key not in _PROG_CACHE:
        _PROG_CACHE[key] = _build_program(k_pad, n_chunks)
    return _PROG_CACHE[key]


# -------------------------------------------------------------------- kernel

def kernel(seg_pred: np.ndarray, bboxes: np.ndarray, batch_idx: np.ndarray,
           is_seg: np.ndarray) -> np.ndarray:
    seg_pred = np.asarray(seg_pred, dtype=np.float32)
    bboxes = np.asarray(bboxes, dtype=np.float32)
    batch_idx = np.asarray(batch_idx)
    is_seg = np.asarray(is_seg).astype(bool)
    assert seg_pred.shape == (B, 1, H, W), seg_pred.shape

    x1, y1, x2, y2 = _box_coords(bboxes, H, W)
    per_img = [[] for _ in range(B)]
    for m in range(bboxes.shape[0]):
        bi = int(batch_idx[m])
        if 0 <= bi < B:
            per_img[bi].append((int(x1[m]), int(y1[m]), int(x2[m]), int(y2[m])))

    has_box = np.array([len(p) > 0 for p in per_img])
    eff = (~is_seg) & has_box
    if not (eff.any() and not is_seg.all()):
        return np.float32(0.0)

    rects = [_disjoint_rects(p) for p in per_img]
    k_max = max(len(r) for r in rects)
    n_chunks = max(1, math.ceil(k_max / PT))
    k_pad = min(PT, max(16, math.ceil(k_max / n_chunks / 16) * 16))
    V = IPC * n_chunks

    # per-core input arrays
    in_maps = []
    for core in range(N_CORES):
        imgs = [core * IPC + i for i in range(IPC)]
        # [i, p, t*W+w] layout: each SBUF partition's payload is contiguous
        seg_arr = np.ascontiguousarray(
            seg_pred[imgs, 0].reshape(IPC, NT, PT, W).transpose(0, 2, 1, 3)
            .reshape(IPC, PT, NT * W).astype(SEG_NP))
        rh_arr = np.zeros((PT, V * NT * k_pad), SEG_NP)
        ch_arr = np.zeros((k_pad, V * W), SEG_NP)
        for i, b in enumerate(imgs):
            for r, (rx1, rx2, ry1, ry2) in enumerate(rects[b]):
                c, rr = divmod(r, k_pad)
                v = i * n_chunks + c
                ch_arr[rr, v * W + rx1:v * W + rx2 + 1] = 1
                for t in range(NT):
                    lo, hi = max(ry1, t * PT), min(ry2, t * PT + PT - 1)
                    if lo <= hi:
                        col = (v * NT + t) * k_pad + rr
                        rh_arr[lo - t * PT:hi - t * PT + 1, col] = 1
        in_maps.append({"seg": seg_arr, "rowhit": rh_arr, "colhit": ch_arr})

    nc = _get_program(k_pad, n_chunks)
    global LAST_RESULT
    res = run_bass_kernel_spmd(nc, in_maps, list(range(N_CORES)), trace=TRACE)
    LAST_RESULT = res

    # host reduction in float64
    total = 0.0
    log2_full = math.log(2.0) * H * W
    for core in range(N_CORES):
        outa = res.results[core]["out"].astype(np.float64)  # [128, IPC+V]
        for i in range(IPC):
            b = core * IPC + i
            if eff[b]:
                m_sum = outa[:k_pad,
                             IPC + i * n_chunks:IPC + (i + 1) * n_chunks].sum()
                total += outa[:, i].sum() - m_sum
            else:
                total += log2_full
    loss = WEIGHT * total / (B * H * W)
    return np.float32(loss)

